# revision 34
# baseline (speedup 1.0000x reference)
"""Cached multi-head attention (decode-append, S=4) on 8 Trainium2 NeuronCores.

Sharding: tensor-parallel over the 32 heads -> 4 heads per core.
  - Wq/Wk/Wv split on the output-feature (head) axis, Wo on the input axis.
  - Each core holds its heads' slice of the KV cache (positions 0..4095; the
    4 new positions are computed on-device from hidden_states).
  - Each core produces a partial [32, 4096] o_proj output; the "all-reduce"
    is a host-side sum of the 8 partials.

KV cache streaming (halves HBM traffic vs fp16; weights/activations fp16):
  - K: int8 with per-position absmax scales, cast to fp16 in-flight by the
    SWDGE DMA (free: the cast rides the descriptor path); the per-position
    scale (x softmax SCALE) is applied to the scores by a DVE multiply with
    a host-precomputed scale map before the exp.
  - V: float8 e3m4 fed directly to the PE as the stationary operand
    (pre-scaled by ALPHA to center the format range; ALPHA is divided back
    out via the denominator broadcast and folded into Wv host-side).

Per-core device kernel:
  phase 1: x-stationary projections -> q/k/v token-major [32, 512], PE
           transposes for feature-major qT/kT; per-batch v_new slices
           (pre-scaled by ALPHA_V via Wv) at partitions 0..3.
  phase 2: per (b, h): scores via K-tile-stationary matmuls (fp8 K x fp16 q),
           exp via ACT (scale=SCALE/ALPHA_K; max-subtraction skipped,
           |scores| <~ 6), new-token scores with causal mask via the fp16
           path. PV with V-tile-stationary matmuls (fp8 V x fp16 probs
           moving) -> feature-major [128, 4] accumulation; softmax
           denominator via a ones-row matmul over probs (+ pn), broadcast
           through an outer-product matmul, reciprocal, and a DVE multiply
           that writes attnT directly (no PE transpose needed).
  phase 3: o_proj with attnT-as-weights -> partial [32, 4096] fp32.
"""

import numpy as np
import ml_dtypes

import concourse.bacc as bacc
import concourse.mybir as mybir
import concourse.tile as tile
from concourse.bass_utils import run_bass_kernel_spmd

N_CORES = 8
B, S, H = 8, 4, 4096
NH = 32                 # total heads
HPC = NH // N_CORES     # heads per core = 4
HD = H // NH            # head dim = 128
POS = 4096              # cache positions attended (rows >= POS are overwritten)
NT = POS // 128         # kv tiles per (b, h) = 32
NTOK = B * S            # 32 query tokens, token index = 4*b + s
KPC = HPC * HD          # per-core feature slice = 512
SCALE = HD ** -0.5
NEG_INF = -1e9
ALPHA = 2.828427        # e3m4 pre-scale (keeps |alpha*v| < 15.5, no clipping)
NI8 = NT // 2           # K tiles per head stored as int8 (rest e3m4-direct)
P8 = NI8 * 128          # int8 positions per head = 2048
PF = (NT - NI8) * 128   # e3m4 positions per head = 2048

F8 = mybir.dt.float8e3
F16 = mybir.dt.float16
F32 = mybir.dt.float32
I8 = mybir.dt.int8
E3M4 = ml_dtypes.float8_e3m4


def build_nc():
    nc = bacc.Bacc("TRN2", target_bir_lowering=False)

    xT = nc.dram_tensor("xT", [128, NT * NTOK], F16, kind="ExternalInput")
    wq = nc.dram_tensor("wq", [128, NT * KPC], F16, kind="ExternalInput")
    wk = nc.dram_tensor("wk", [128, NT * KPC], F16, kind="ExternalInput")
    wv = nc.dram_tensor("wv", [128, NT * KPC], F16, kind="ExternalInput")
    wo = nc.dram_tensor("wo", [128, HPC * H], F16, kind="ExternalInput")
    kt = nc.dram_tensor("kt", [B, 128, HPC * P8], I8, kind="ExternalInput")
    ktf = nc.dram_tensor("ktf", [B, 128, HPC * PF], F8, kind="ExternalInput")
    v = nc.dram_tensor("v", [B, 128, HPC * NT * HD], F8, kind="ExternalInput")
    cmap = nc.dram_tensor("cmap", [128, B * HPC * NT * S], F16, kind="ExternalInput")
    mask = nc.dram_tensor("mask", [S, S], F32, kind="ExternalInput")
    ident = nc.dram_tensor("ident", [32, 32], F16, kind="ExternalInput")
    out = nc.dram_tensor("out", [NTOK, H], F16, kind="ExternalOutput")

    with tile.TileContext(nc) as tc:
        _body(tc, xT.ap(), wq.ap(), wk.ap(), wv.ap(), wo.ap(), kt.ap(),
              ktf.ap(), v.ap(), cmap.ap(), mask.ap(), ident.ap(), out.ap())
    nc.compile()
    return nc


def _body(tc, xT, wq, wk, wv, wo, kt, ktf, v, cmap, mask, ident, out):
    nc = tc.nc
    from contextlib import ExitStack
    Exp = mybir.ActivationFunctionType.Exp
    HT = NT // 2
    ctx = ExitStack()
    with ctx:
        consts = ctx.enter_context(tc.tile_pool(name="consts", bufs=1))
        persist = ctx.enter_context(tc.tile_pool(name="persist", bufs=1))
        wpool = ctx.enter_context(tc.tile_pool(name="wpool", bufs=2))
        kvpool = ctx.enter_context(tc.tile_pool(name="kvpool", bufs=4))
        smpool = ctx.enter_context(tc.tile_pool(name="smpool", bufs=2))
        ps = ctx.enter_context(tc.tile_pool(name="ps", bufs=2, space="PSUM"))

        # ---- DMA preamble: interleave first kv chunks with weight halves ----
        xT_sb = persist.tile([128, NT * NTOK], F16)
        nc.sync.dma_start(out=xT_sb, in_=xT)
        mask_sb = consts.tile([S, S], F32)
        nc.sync.dma_start(out=mask_sb, in_=mask)
        id_sb = consts.tile([32, 32], F16)
        nc.sync.dma_start(out=id_sb, in_=ident)
        cmap_sb = persist.tile([128, B * HPC * NT * S], F16)
        ones_col = consts.tile([128, 1], F16)
        nc.vector.memset(ones_col, 1.0)
        a_ones = consts.tile([1, 128], F16)
        nc.vector.memset(a_ones, ALPHA)

        def w_halves(w_dram, name):
            tiles = []
            for half in range(2):
                wh = wpool.tile([128, HT * KPC], F16, tag="w", name=f"{name}{half}")
                nc.sync.dma_start(
                    out=wh, in_=w_dram[:, HT * KPC * half: HT * KPC * (half + 1)])
                tiles.append(wh)
            return tiles

        kvch = {}

        def fetch_kv(b, hp):
            # K splits into an int8 part (cast to fp16 by the ACT engine,
            # per-position scales via cmap) and an e3m4 part fed to the PE
            # directly; both are 1 B/elem on the wire. Fractions sized so the
            # ACT engine keeps slack vs the chunk cadence.
            kt8 = kvpool.tile([128, 2 * P8], I8, tag="kt8", name=f"kt8_{b}{hp}",
                              bufs=6)
            nc.sync.dma_start(out=kt8, in_=kt[b][:, 2 * P8 * hp: 2 * P8 * (hp + 1)])
            ktf8 = kvpool.tile([128, 2 * PF], F8, tag="ktf", name=f"ktf{b}{hp}",
                               bufs=6)
            nc.sync.dma_start(out=ktf8, in_=ktf[b][:, 2 * PF * hp: 2 * PF * (hp + 1)])
            vch = kvpool.tile([128, 2 * NT * HD], F8, tag="v", name=f"v{b}{hp}",
                              bufs=6)
            nc.gpsimd.dma_start(out=vch, in_=v[b][:, 2 * NT * HD * hp: 2 * NT * HD * (hp + 1)])
            kvch[(b, hp)] = (kt8, ktf8, vch)

        def cast_half(kt8, ktch, hh):
            o = P8 * hh
            for lo, hi in ((0, P8 // 2), (P8 // 2, P8)):
                nc.scalar.copy(out=ktch[:, o + lo: o + hi],
                               in_=kt8[:, o + lo: o + hi])

        wqh = w_halves(wq, "wq")
        fetch_kv(0, 0)
        nc.sync.dma_start(out=cmap_sb, in_=cmap)
        wkh = w_halves(wk, "wk")
        fetch_kv(1, 0)
        wvh = w_halves(wv, "wv")
        fetch_kv(2, 0)
        fetch_kv(3, 0)
        # o_proj weights on the SWDGE ring, overlapping the attention stream
        wo_a = wpool.tile([128, 2 * H], F16, tag="w")
        nc.gpsimd.dma_start(out=wo_a, in_=wo[:, 0: 2 * H])
        wo_b = wpool.tile([128, 2 * H], F16, tag="w")
        nc.gpsimd.dma_start(out=wo_b, in_=wo[:, 2 * H: 4 * H])

        # ---- phase 1: projections (x-stationary, token-major) ----
        qT_sb = persist.tile([128, HPC * NTOK], F16)
        kT_sb = persist.tile([128, HPC * NTOK], F16)
        attnT_sb = persist.tile([128, HPC * NTOK], F16)
        vnew_sb = [persist.tile([S, KPC], F16, name=f"vnew{b}") for b in range(B)]

        q_tok = persist.tile([NTOK, KPC], F16)
        k_tok = persist.tile([NTOK, KPC], F16)
        v_tok = persist.tile([NTOK, KPC], F16)

        def proj(whs, tok_dst, tagp, nbufs):
            pp = ps.tile([NTOK, KPC], F32, tag=tagp, name=f"pp_{tagp}",
                         bufs=nbufs)
            for half in range(2):
                for tt in range(HT):
                    t = HT * half + tt
                    nc.tensor.matmul(
                        pp, lhsT=xT_sb[:, NTOK * t: NTOK * (t + 1)],
                        rhs=whs[half][:, KPC * tt: KPC * (tt + 1)],
                        start=(t == 0), stop=(t == NT - 1))
            nc.scalar.copy(out=tok_dst, in_=pp)

        def transp(src_t, dst):
            for m in range(HPC):
                tp = ps.tile([128, NTOK], F16, tag="pv", bufs=2)
                nc.tensor.transpose(tp, in_=src_t[:, HD * m: HD * (m + 1)], identity=id_sb)
                nc.scalar.copy(out=dst[:, NTOK * m: NTOK * (m + 1)], in_=tp)

        # ---- phase 2: attention (head-pair major: o_proj can start halfway) ----
        o_part = persist.tile([NTOK, H], F16)
        o_all = persist.tile([NTOK, H], F16)
        jobs = []
        for hp in range(HPC // 2):
            for b in range(B):
                for hh in range(2):
                    jobs.append((b, hp, hh))
        NJ = len(jobs)
        kt16 = {}

        def emit_cast(i):
            """Fetch (if needed) + emit the fp16 cast for job i's K half.

            Called one job ahead of processing so the ACT engine casts the
            next head's K while the PE/DVE work on the current head — the
            exp never queues behind a cast it doesn't depend on."""
            if i >= NJ:
                return
            b, hp, hh = jobs[i]
            if (b, hp) not in kvch:
                fetch_kv(b, hp)
            if (b, hp) not in kt16:
                kt16[(b, hp)] = kvpool.tile([128, 2 * P8], F16, tag="kt",
                                            name=f"kt{b}{hp}", bufs=3)
            cast_half(kvch[(b, hp)][0], kt16[(b, hp)], hh)

        # q first: qT is the only phase-1 artifact the scores stream needs;
        # then prime two K casts so the ACT engine starts the moment kt8
        # lands, and finish the k/v projections behind them
        proj(wqh, q_tok, "scores", 3)
        transp(q_tok, qT_sb)
        emit_cast(0)
        emit_cast(1)
        proj(wkh, k_tok, "pv", 2)
        transp(k_tok, kT_sb)
        proj(wvh, v_tok, "small", 2)
        # per-batch v_new [4, 512] (already ALPHA-scaled via Wv) at parts 0..3
        for b in range(B):
            nc.gpsimd.dma_start(out=vnew_sb[b], in_=v_tok[S * b: S * (b + 1), :])

        for i, (b, hp, hh) in enumerate(jobs):
            h = 2 * hp + hh
            ktch, ktf8ch, vch = kt16[(b, hp)], kvch[(b, hp)][1], kvch[(b, hp)][2]
            koff, foff, voff = P8 * hh, PF * hh, NT * HD * hh
            if True:
                if True:
                    col = NTOK * h + S * b  # (head, batch) column in qT/kT/attnT
                    scores = ps.tile([128, NT * S], F32, tag="scores", bufs=3)
                    for t in range(NT):
                        if t < NI8:
                            lh = ktch[:, koff + 128 * t: koff + 128 * t + 128]
                        else:
                            tf = t - NI8
                            lh = ktf8ch[:, foff + 128 * tf: foff + 128 * tf + 128]
                        nc.tensor.matmul(
                            scores[:, S * t: S * (t + 1)], lhsT=lh,
                            rhs=qT_sb[:, col: col + S],
                            start=True, stop=True,
                        )
                    emit_cast(i + 2)
                    # apply the per-position int8 K scales (x softmax SCALE)
                    coff = (b * HPC + h) * NT * S
                    nc.vector.tensor_mul(out=scores, in0=scores,
                                         in1=cmap_sb[:, coff: coff + NT * S])
                    probs = smpool.tile([128, NT * S], F16, tag="probs")
                    nc.scalar.activation(out=probs, in_=scores, func=Exp,
                                         scale=1.0)
                    # new-token scores [4 kv_new, 4 tok] + causal mask (separate
                    # tiles so the cache pipeline doesn't wait on k/v proj)
                    sn = ps.tile([S, S], F32, tag="small", bufs=2)
                    nc.tensor.matmul(sn, lhsT=kT_sb[:, col: col + S],
                                     rhs=qT_sb[:, col: col + S], start=True, stop=True)
                    nc.vector.tensor_add(out=sn, in0=sn, in1=mask_sb)
                    pn = smpool.tile([S, S], F16, tag="pn")
                    nc.scalar.activation(out=pn, in_=sn, func=Exp, scale=SCALE)
                    # PV: V-tile stationary (fp8), probs moving -> feature-major
                    opv = ps.tile([128, S], F32, tag="pv", bufs=2)
                    for t in range(NT):
                        nc.tensor.matmul(
                            opv,
                            lhsT=vch[:, voff + HD * t: voff + HD * (t + 1)],
                            rhs=probs[:, S * t: S * (t + 1)],
                            start=(t == 0), stop=False,
                        )
                    nc.tensor.matmul(
                        opv, lhsT=vnew_sb[b][:, HD * h: HD * (h + 1)], rhs=pn,
                        start=False, stop=True,
                    )
                    # softmax denominator: ones-row matmuls over probs (s-major
                    # stream) and pn, reduced + broadcast via outer product
                    den = ps.tile([1, S * NT], F32, tag="small", bufs=2)
                    nc.tensor.matmul(
                        den, lhsT=ones_col,
                        rhs=probs.rearrange("p (t s) -> p s t", s=S),
                        start=True, stop=True)
                    den2 = ps.tile([1, S], F32, tag="small", bufs=2)
                    nc.tensor.matmul(den2, lhsT=ones_col[0:S, 0:1], rhs=pn,
                                     start=True, stop=True)
                    denr = smpool.tile([1, S], F32, tag="denr")
                    nc.vector.tensor_reduce(
                        out=denr, in_=den.rearrange("p (s t) -> p s t", s=S),
                        axis=mybir.AxisListType.X, op=mybir.AluOpType.add)
                    den16 = smpool.tile([1, S], F16, tag="den16")
                    nc.vector.tensor_add(out=den16, in0=denr, in1=den2)
                    dbc = ps.tile([128, S], F32, tag="small", bufs=2)
                    nc.tensor.matmul(dbc, lhsT=a_ones, rhs=den16,
                                     start=True, stop=True)
                    recb = smpool.tile([128, S], F32, tag="recb")
                    nc.vector.reciprocal(out=recb, in_=dbc)
                    nc.vector.tensor_mul(out=attnT_sb[:, col: col + S],
                                         in0=opv, in1=recb)

            if b == B - 1 and hh == 1:
                # o_proj for this head pair: hp==0 stages into o_part (DVE
                # copy, keeping the ACT engine free for casts), hp==1 adds
                for n in range(H // 512):
                    op = ps.tile([NTOK, 512], F32, tag="scores", bufs=3)
                    for jj in range(2):
                        j = 2 * hp + jj
                        wo_half = wo_a if hp == 0 else wo_b
                        nc.tensor.matmul(
                            op,
                            lhsT=attnT_sb[:, NTOK * j: NTOK * (j + 1)],
                            rhs=wo_half[:, H * jj + 512 * n: H * jj + 512 * (n + 1)],
                            start=(jj == 0), stop=(jj == 1),
                        )
                    if hp == 0:
                        nc.vector.tensor_copy(out=o_part[:, 512 * n: 512 * (n + 1)],
                                              in_=op)
                    else:
                        nc.vector.tensor_add(out=o_all[:, 512 * n: 512 * (n + 1)],
                                             in0=op,
                                             in1=o_part[:, 512 * n: 512 * (n + 1)])
                        nc.sync.dma_start(out=out[:, 512 * n: 512 * (n + 1)],
                                          in_=o_all[:, 512 * n: 512 * (n + 1)])


# ---------------------------------------------------------------------------
# host side
# ---------------------------------------------------------------------------

def build_core_inputs(hidden_states, Wq, Wk, Wv, Wo, key_cache, value_cache):
    """Shard + lay out the full inputs into the 8 per-core DRAM images."""
    tokens = np.ascontiguousarray(hidden_states.reshape(NTOK, H))
    xT = tokens.T.astype(np.float16)                       # [4096, 32]
    xT_sb = np.ascontiguousarray(
        xT.reshape(NT, 128, NTOK).transpose(1, 0, 2)).reshape(128, NT * NTOK)

    WqT = Wq.T.astype(np.float16)                          # [in=4096, out=4096]
    WkT = Wk.T.astype(np.float16)
    WvT = (Wv.T * np.float32(ALPHA)).astype(np.float16)    # ALPHA folded into v_new
    WoT = Wo.T.astype(np.float16)                          # [in, out]
    Kf = key_cache[:, :, :POS, :].astype(np.float32)       # [B, NH, POS, HD]
    K8p = Kf[:, :, :P8, :]                                 # int8 part
    csc = np.abs(K8p).max(axis=-1, keepdims=True) * np.float32(1.0 / 127.0)
    K8 = np.round(K8p / csc).astype(np.int8)
    KF8 = (Kf[:, :, P8:, :] * np.float32(ALPHA)).astype(E3M4)  # e3m4 part
    V8 = (value_cache[:, :, :POS, :] * np.float32(ALPHA)).astype(E3M4)

    mask = np.where(np.arange(S)[:, None] > np.arange(S)[None, :],
                    np.float32(NEG_INF), np.float32(0.0))
    ident = np.eye(32, dtype=np.float16)

    in_maps = []
    for c in range(N_CORES):
        cs = slice(KPC * c, KPC * (c + 1))
        hs = slice(HPC * c, HPC * (c + 1))

        def wlayout(WT):
            a = np.ascontiguousarray(WT[:, cs])            # [4096, 512]
            return np.ascontiguousarray(
                a.reshape(NT, 128, KPC).transpose(1, 0, 2)).reshape(128, NT * KPC)

        wo_c = np.ascontiguousarray(WoT[cs, :])            # [512, 4096]
        wo_c = np.ascontiguousarray(
            wo_c.reshape(HPC, 128, H).transpose(1, 0, 2)).reshape(128, HPC * H)

        kt_c = np.ascontiguousarray(
            K8[:, hs].transpose(0, 3, 1, 2)).reshape(B, 128, HPC * P8)
        ktf_c = np.ascontiguousarray(
            KF8[:, hs].transpose(0, 3, 1, 2)).reshape(B, 128, HPC * PF)
        v_p = V8[:, hs].reshape(B, HPC, NT, 128, HD)       # [b, h, t, kv, d]
        v_c = np.ascontiguousarray(
            v_p.transpose(0, 3, 1, 2, 4)).reshape(B, 128, HPC * NT * HD)

        # cmap[p, ((b*HPC+h)*NT + t)*S + s]: int8 tiles get SCALE * c[...],
        # e3m4 tiles get the constant SCALE / ALPHA
        c_r = (csc[:, hs, :, 0] * np.float32(SCALE)).reshape(B, HPC, NI8, 128)
        c_full = np.full((B, HPC, NT, 128), np.float32(SCALE / ALPHA))
        c_full[:, :, :NI8, :] = c_r
        c_t = np.ascontiguousarray(c_full.transpose(3, 0, 1, 2))
        cmap_c = np.broadcast_to(
            c_t[..., None], (128, B, HPC, NT, S))
        cmap_c = np.ascontiguousarray(cmap_c).reshape(
            128, B * HPC * NT * S).astype(np.float16)

        in_maps.append({
            "xT": xT_sb, "wq": wlayout(WqT), "wk": wlayout(WkT),
            "wv": wlayout(WvT), "wo": wo_c, "kt": kt_c, "ktf": ktf_c,
            "v": v_c, "cmap": cmap_c, "mask": mask, "ident": ident,
        })
    return in_maps


def numpy_core_kernel(m):
    """Numpy mirror of the device dataflow for one core (layout validation)."""
    f = np.float32
    f16 = np.float16
    xT_sb = m["xT"].astype(f)
    xT = xT_sb.reshape(128, NT, NTOK).transpose(1, 0, 2).reshape(H, NTOK)

    def unw(w):
        return w.astype(f).reshape(128, NT, KPC).transpose(1, 0, 2).reshape(H, KPC)

    qT = (unw(m["wq"]).T @ xT).astype(f16).astype(f)      # [512 feat, 32 tok]
    kT = (unw(m["wk"]).T @ xT).astype(f16).astype(f)
    vnew = (unw(m["wv"]).T @ xT).T.astype(f16).astype(f)  # [32 tok, 512 feat], x ALPHA

    attnT = np.zeros((KPC, NTOK), f)
    for b in range(B):
        for h in range(HPC):
            colsl = slice(S * b, S * b + S)
            K8bh = m["kt"][b].astype(f)[:, P8 * h: P8 * (h + 1)]     # [hd, p8] int8
            KFbh = m["ktf"][b].astype(f)[:, PF * h: PF * (h + 1)]    # [hd, pf] e3m4
            KTbh = np.concatenate([K8bh, KFbh], axis=1)              # [hd, kv]
            scoresT = KTbh.T @ qT[HD * h: HD * (h + 1), colsl]       # [kv, 4]
            coff = (b * HPC + h) * NT * S
            cm = m["cmap"][:, coff: coff + NT * S].astype(f)         # [128, NT*S]
            cm_kv = cm.reshape(128, NT, S).transpose(1, 0, 2).reshape(POS, S)
            scoresT = scoresT * cm_kv
            snew = kT[HD * h: HD * (h + 1), colsl].T @ qT[HD * h: HD * (h + 1), colsl]
            snew = snew + m["mask"]                                  # [j, s]
            pr = np.exp(scoresT).astype(f16).astype(f)
            prnew = np.exp(SCALE * snew).astype(f16).astype(f)
            den = (pr.sum(axis=0) + prnew.sum(axis=0)).astype(f16).astype(f)
            vb = m["v"][b].astype(f)[:, NT * HD * h: NT * HD * (h + 1)]  # x ALPHA
            V_bh = vb.reshape(128, NT, HD).transpose(1, 0, 2).reshape(POS, HD)
            ou = V_bh.T @ pr + vnew[S * b: S * b + S, HD * h: HD * (h + 1)].T @ prnew
            attnT[HD * h: HD * (h + 1), colsl] = (ou / (ALPHA * den)).astype(f16)
    woc = m["wo"].astype(f).reshape(128, HPC, H).transpose(1, 0, 2).reshape(KPC, H)
    return (attnT.astype(f16).astype(f).T @ woc).astype(np.float16).astype(np.float32)


_NC_CACHE = None


def get_nc():
    global _NC_CACHE
    if _NC_CACHE is None:
        _NC_CACHE = build_nc()
    return _NC_CACHE


def run_on_hw(inputs, trace=False, trace_cores=None):
    position = int(inputs["position"])
    assert position == POS, position
    in_maps = build_core_inputs(
        np.asarray(inputs["hidden_states"]), np.asarray(inputs["Wq"]),
        np.asarray(inputs["Wk"]), np.asarray(inputs["Wv"]), np.asarray(inputs["Wo"]),
        np.asarray(inputs["key_cache"]), np.asarray(inputs["value_cache"]))
    nc = get_nc()
    res = run_bass_kernel_spmd(nc, in_maps, core_ids=list(range(N_CORES)),
                               trace=trace, trace_cores=trace_cores)
    partial = np.zeros((NTOK, H), np.float64)
    for c in range(N_CORES):
        partial += res.results[c]["out"].astype(np.float64)
    out = partial.astype(np.float32).reshape(B, S, H)
    return out, res


def kernel(**inputs) -> np.ndarray:
    out, _ = run_on_hw(inputs, trace=False)
    return out


# revision 35
# speedup vs baseline: 1.0799x; 1.0799x over previous
"""Cached multi-head attention (decode-append, S=4) on 8 Trainium2 NeuronCores.

Sharding: tensor-parallel over the 32 heads -> 4 heads per core.
  - Wq/Wk/Wv split on the output-feature (head) axis, Wo on the input axis.
  - Each core holds its heads' slice of the KV cache (positions 0..4095; the
    4 new positions are computed on-device from hidden_states).
  - Each core produces a partial [32, 4096] o_proj output; the "all-reduce"
    is a host-side sum of the 8 partials.

KV cache streaming (halves HBM traffic vs fp16; weights/activations fp16):
  - K: int8 with per-position absmax scales, cast to fp16 in-flight by the
    SWDGE DMA (free: the cast rides the descriptor path); the per-position
    scale (x softmax SCALE) is applied to the scores by a DVE multiply with
    a host-precomputed scale map before the exp.
  - V: float8 e3m4 fed directly to the PE as the stationary operand
    (pre-scaled by ALPHA to center the format range; ALPHA is divided back
    out via the denominator broadcast and folded into Wv host-side).

Per-core device kernel:
  phase 1: x-stationary projections -> q/k/v token-major [32, 512], PE
           transposes for feature-major qT/kT; per-batch v_new slices
           (pre-scaled by ALPHA_V via Wv) at partitions 0..3.
  phase 2: per (b, h): scores via K-tile-stationary matmuls (fp8 K x fp16 q),
           exp via ACT (scale=SCALE/ALPHA_K; max-subtraction skipped,
           |scores| <~ 6), new-token scores with causal mask via the fp16
           path. PV with V-tile-stationary matmuls (fp8 V x fp16 probs
           moving) -> feature-major [128, 4] accumulation; softmax
           denominator via a ones-row matmul over probs (+ pn), broadcast
           through an outer-product matmul, reciprocal, and a DVE multiply
           that writes attnT directly (no PE transpose needed).
  phase 3: o_proj with attnT-as-weights -> partial [32, 4096] fp32.
"""

import numpy as np
import ml_dtypes

import concourse.bacc as bacc
import concourse.mybir as mybir
import concourse.tile as tile
from concourse.bass_utils import run_bass_kernel_spmd

N_CORES = 8
B, S, H = 8, 4, 4096
NH = 32                 # total heads
HPC = NH // N_CORES     # heads per core = 4
HD = H // NH            # head dim = 128
POS = 4096              # cache positions attended (rows >= POS are overwritten)
NT = POS // 128         # kv tiles per (b, h) = 32
NTOK = B * S            # 32 query tokens, token index = 4*b + s
KPC = HPC * HD          # per-core feature slice = 512
SCALE = HD ** -0.5
NEG_INF = -1e9
ALPHA = 2.828427        # e3m4 pre-scale (keeps |alpha*v| < 15.5, no clipping)
NI8 = NT // 2           # K tiles per head stored as int8 (rest e3m4-direct)
P8 = NI8 * 128          # int8 positions per head = 2048
PF = (NT - NI8) * 128   # e3m4 positions per head = 2048

F8 = mybir.dt.float8e3
F16 = mybir.dt.float16
F32 = mybir.dt.float32
I8 = mybir.dt.int8
E3M4 = ml_dtypes.float8_e3m4


def build_nc():
    nc = bacc.Bacc("TRN2", target_bir_lowering=False)

    xT = nc.dram_tensor("xT", [128, NT * NTOK], F16, kind="ExternalInput")
    wq = nc.dram_tensor("wq", [128, NT * KPC], F16, kind="ExternalInput")
    wk = nc.dram_tensor("wk", [128, NT * KPC], F16, kind="ExternalInput")
    wv = nc.dram_tensor("wv", [128, NT * KPC], F16, kind="ExternalInput")
    wo = nc.dram_tensor("wo", [128, HPC * H], F16, kind="ExternalInput")
    kt = nc.dram_tensor("kt", [B, 128, HPC * P8], I8, kind="ExternalInput")
    ktf = nc.dram_tensor("ktf", [B, 128, HPC * PF], F8, kind="ExternalInput")
    v = nc.dram_tensor("v", [B, 128, HPC * NT * HD], F8, kind="ExternalInput")
    cmap = nc.dram_tensor("cmap", [128, B * HPC * NT * S], F16, kind="ExternalInput")
    mask = nc.dram_tensor("mask", [S, S], F32, kind="ExternalInput")
    ident = nc.dram_tensor("ident", [32, 32], F16, kind="ExternalInput")
    out = nc.dram_tensor("out", [NTOK, H], F16, kind="ExternalOutput")

    with tile.TileContext(nc) as tc:
        _body(tc, xT.ap(), wq.ap(), wk.ap(), wv.ap(), wo.ap(), kt.ap(),
              ktf.ap(), v.ap(), cmap.ap(), mask.ap(), ident.ap(), out.ap())
    nc.compile()
    return nc


def _body(tc, xT, wq, wk, wv, wo, kt, ktf, v, cmap, mask, ident, out):
    nc = tc.nc
    from contextlib import ExitStack
    Exp = mybir.ActivationFunctionType.Exp
    HT = NT // 2
    ctx = ExitStack()
    with ctx:
        consts = ctx.enter_context(tc.tile_pool(name="consts", bufs=1))
        persist = ctx.enter_context(tc.tile_pool(name="persist", bufs=1))
        wpool = ctx.enter_context(tc.tile_pool(name="wpool", bufs=2))
        kvpool = ctx.enter_context(tc.tile_pool(name="kvpool", bufs=4))
        smpool = ctx.enter_context(tc.tile_pool(name="smpool", bufs=2))
        ps = ctx.enter_context(tc.tile_pool(name="ps", bufs=2, space="PSUM"))

        # ---- DMA preamble: interleave first kv chunks with weight halves ----
        xT_sb = persist.tile([128, NT * NTOK], F16)
        nc.sync.dma_start(out=xT_sb, in_=xT)
        mask_sb = consts.tile([S, S], F32)
        nc.sync.dma_start(out=mask_sb, in_=mask)
        id_sb = consts.tile([32, 32], F16)
        nc.sync.dma_start(out=id_sb, in_=ident)
        cmap_sb = persist.tile([128, B * HPC * NT * S], F16)
        ones_col = consts.tile([128, 1], F16)
        nc.vector.memset(ones_col, 1.0)
        a_ones = consts.tile([1, 128], F16)
        nc.vector.memset(a_ones, ALPHA)

        def w_halves(w_dram, name):
            tiles = []
            for half in range(2):
                wh = wpool.tile([128, HT * KPC], F16, tag="w", name=f"{name}{half}")
                nc.sync.dma_start(
                    out=wh, in_=w_dram[:, HT * KPC * half: HT * KPC * (half + 1)])
                tiles.append(wh)
            return tiles

        kvch = {}

        def fetch_kv(b, hp):
            # K splits into an int8 part (cast to fp16 by the ACT engine,
            # per-position scales via cmap) and an e3m4 part fed to the PE
            # directly; both are 1 B/elem on the wire. Fractions sized so the
            # ACT engine keeps slack vs the chunk cadence.
            kt8 = kvpool.tile([128, 2 * P8], I8, tag="kt8", name=f"kt8_{b}{hp}",
                              bufs=6)
            nc.sync.dma_start(out=kt8, in_=kt[b][:, 2 * P8 * hp: 2 * P8 * (hp + 1)])
            ktf8 = kvpool.tile([128, 2 * PF], F8, tag="ktf", name=f"ktf{b}{hp}",
                               bufs=6)
            nc.sync.dma_start(out=ktf8, in_=ktf[b][:, 2 * PF * hp: 2 * PF * (hp + 1)])
            vch = kvpool.tile([128, 2 * NT * HD], F8, tag="v", name=f"v{b}{hp}",
                              bufs=6)
            nc.sync.dma_start(out=vch, in_=v[b][:, 2 * NT * HD * hp: 2 * NT * HD * (hp + 1)])
            kvch[(b, hp)] = (kt8, ktf8, vch)

        def cast_half(kt8, ktch, hh):
            o = P8 * hh
            for lo, hi in ((0, P8 // 2), (P8 // 2, P8)):
                nc.scalar.copy(out=ktch[:, o + lo: o + hi],
                               in_=kt8[:, o + lo: o + hi])

        wqh = w_halves(wq, "wq")
        fetch_kv(0, 0)
        nc.sync.dma_start(out=cmap_sb, in_=cmap)
        wkh = w_halves(wk, "wk")
        fetch_kv(1, 0)
        wvh = w_halves(wv, "wv")
        fetch_kv(2, 0)
        fetch_kv(3, 0)
        # o_proj weights on the SWDGE ring, overlapping the attention stream
        wo_a = wpool.tile([128, 2 * H], F16, tag="w")
        nc.gpsimd.dma_start(out=wo_a, in_=wo[:, 0: 2 * H])
        wo_b = wpool.tile([128, 2 * H], F16, tag="w")
        nc.gpsimd.dma_start(out=wo_b, in_=wo[:, 2 * H: 4 * H])

        # ---- phase 1: projections (x-stationary, token-major) ----
        qT_sb = persist.tile([128, HPC * NTOK], F16)
        kT_sb = persist.tile([128, HPC * NTOK], F16)
        attnT_sb = persist.tile([128, HPC * NTOK], F16)
        vnew_sb = [persist.tile([S, KPC], F16, name=f"vnew{b}") for b in range(B)]

        q_tok = persist.tile([NTOK, KPC], F16)
        k_tok = persist.tile([NTOK, KPC], F16)
        v_tok = persist.tile([NTOK, KPC], F16)

        def proj(whs, tok_dst, tagp, nbufs):
            pp = ps.tile([NTOK, KPC], F32, tag=tagp, name=f"pp_{tagp}",
                         bufs=nbufs)
            for half in range(2):
                for tt in range(HT):
                    t = HT * half + tt
                    nc.tensor.matmul(
                        pp, lhsT=xT_sb[:, NTOK * t: NTOK * (t + 1)],
                        rhs=whs[half][:, KPC * tt: KPC * (tt + 1)],
                        start=(t == 0), stop=(t == NT - 1))
            nc.scalar.copy(out=tok_dst, in_=pp)

        def transp(src_t, dst):
            for m in range(HPC):
                tp = ps.tile([128, NTOK], F16, tag="pv", bufs=2)
                nc.tensor.transpose(tp, in_=src_t[:, HD * m: HD * (m + 1)], identity=id_sb)
                nc.scalar.copy(out=dst[:, NTOK * m: NTOK * (m + 1)], in_=tp)

        # ---- phase 2: attention (head-pair major: o_proj can start halfway) ----
        o_part = persist.tile([NTOK, H], F16)
        o_all = persist.tile([NTOK, H], F16)
        jobs = []
        for hp in range(HPC // 2):
            for b in range(B):
                for hh in range(2):
                    jobs.append((b, hp, hh))
        NJ = len(jobs)
        kt16 = {}

        def emit_cast(i):
            """Fetch (if needed) + emit the fp16 cast for job i's K half.

            Called one job ahead of processing so the ACT engine casts the
            next head's K while the PE/DVE work on the current head — the
            exp never queues behind a cast it doesn't depend on."""
            if i >= NJ:
                return
            b, hp, hh = jobs[i]
            if (b, hp) not in kvch:
                fetch_kv(b, hp)
            if (b, hp) not in kt16:
                kt16[(b, hp)] = kvpool.tile([128, 2 * P8], F16, tag="kt",
                                            name=f"kt{b}{hp}", bufs=3)
            cast_half(kvch[(b, hp)][0], kt16[(b, hp)], hh)

        # q first: qT is the only phase-1 artifact the scores stream needs;
        # then prime two K casts so the ACT engine starts the moment kt8
        # lands, and finish the k/v projections behind them
        proj(wqh, q_tok, "scores", 3)
        transp(q_tok, qT_sb)
        emit_cast(0)
        emit_cast(1)
        proj(wkh, k_tok, "pv", 2)
        transp(k_tok, kT_sb)
        proj(wvh, v_tok, "small", 2)
        # per-batch v_new [4, 512] (already ALPHA-scaled via Wv) at parts 0..3
        for b in range(B):
            nc.gpsimd.dma_start(out=vnew_sb[b], in_=v_tok[S * b: S * (b + 1), :])

        for i, (b, hp, hh) in enumerate(jobs):
            h = 2 * hp + hh
            ktch, ktf8ch, vch = kt16[(b, hp)], kvch[(b, hp)][1], kvch[(b, hp)][2]
            koff, foff, voff = P8 * hh, PF * hh, NT * HD * hh
            if True:
                if True:
                    col = NTOK * h + S * b  # (head, batch) column in qT/kT/attnT
                    scores = ps.tile([128, NT * S], F32, tag="scores", bufs=3)
                    for t in range(NT):
                        if t < NI8:
                            lh = ktch[:, koff + 128 * t: koff + 128 * t + 128]
                        else:
                            tf = t - NI8
                            lh = ktf8ch[:, foff + 128 * tf: foff + 128 * tf + 128]
                        nc.tensor.matmul(
                            scores[:, S * t: S * (t + 1)], lhsT=lh,
                            rhs=qT_sb[:, col: col + S],
                            start=True, stop=True,
                        )
                    emit_cast(i + 2)
                    # apply the per-position int8 K scales (x softmax SCALE)
                    coff = (b * HPC + h) * NT * S
                    nc.vector.tensor_mul(out=scores, in0=scores,
                                         in1=cmap_sb[:, coff: coff + NT * S])
                    probs = smpool.tile([128, NT * S], F16, tag="probs")
                    nc.scalar.activation(out=probs, in_=scores, func=Exp,
                                         scale=1.0)
                    # new-token scores [4 kv_new, 4 tok] + causal mask (separate
                    # tiles so the cache pipeline doesn't wait on k/v proj)
                    sn = ps.tile([S, S], F32, tag="small", bufs=2)
                    nc.tensor.matmul(sn, lhsT=kT_sb[:, col: col + S],
                                     rhs=qT_sb[:, col: col + S], start=True, stop=True)
                    nc.vector.tensor_add(out=sn, in0=sn, in1=mask_sb)
                    pn = smpool.tile([S, S], F16, tag="pn")
                    nc.scalar.activation(out=pn, in_=sn, func=Exp, scale=SCALE)
                    # PV: V-tile stationary (fp8), probs moving -> feature-major
                    opv = ps.tile([128, S], F32, tag="pv", bufs=2)
                    for t in range(NT):
                        nc.tensor.matmul(
                            opv,
                            lhsT=vch[:, voff + HD * t: voff + HD * (t + 1)],
                            rhs=probs[:, S * t: S * (t + 1)],
                            start=(t == 0), stop=False,
                        )
                    nc.tensor.matmul(
                        opv, lhsT=vnew_sb[b][:, HD * h: HD * (h + 1)], rhs=pn,
                        start=False, stop=True,
                    )
                    # softmax denominator: ones-row matmuls over probs (s-major
                    # stream) and pn, reduced + broadcast via outer product
                    den = ps.tile([1, S * NT], F32, tag="small", bufs=2)
                    nc.tensor.matmul(
                        den, lhsT=ones_col,
                        rhs=probs.rearrange("p (t s) -> p s t", s=S),
                        start=True, stop=True)
                    den2 = ps.tile([1, S], F32, tag="small", bufs=2)
                    nc.tensor.matmul(den2, lhsT=ones_col[0:S, 0:1], rhs=pn,
                                     start=True, stop=True)
                    denr = smpool.tile([1, S], F32, tag="denr")
                    nc.vector.tensor_reduce(
                        out=denr, in_=den.rearrange("p (s t) -> p s t", s=S),
                        axis=mybir.AxisListType.X, op=mybir.AluOpType.add)
                    den16 = smpool.tile([1, S], F16, tag="den16")
                    nc.vector.tensor_add(out=den16, in0=denr, in1=den2)
                    dbc = ps.tile([128, S], F32, tag="small", bufs=2)
                    nc.tensor.matmul(dbc, lhsT=a_ones, rhs=den16,
                                     start=True, stop=True)
                    recb = smpool.tile([128, S], F32, tag="recb")
                    nc.vector.reciprocal(out=recb, in_=dbc)
                    nc.vector.tensor_mul(out=attnT_sb[:, col: col + S],
                                         in0=opv, in1=recb)

            if b == B - 1 and hh == 1:
                # o_proj for this head pair: hp==0 stages into o_part (DVE
                # copy, keeping the ACT engine free for casts), hp==1 adds
                for n in range(H // 512):
                    op = ps.tile([NTOK, 512], F32, tag="scores", bufs=3)
                    for jj in range(2):
                        j = 2 * hp + jj
                        wo_half = wo_a if hp == 0 else wo_b
                        nc.tensor.matmul(
                            op,
                            lhsT=attnT_sb[:, NTOK * j: NTOK * (j + 1)],
                            rhs=wo_half[:, H * jj + 512 * n: H * jj + 512 * (n + 1)],
                            start=(jj == 0), stop=(jj == 1),
                        )
                    if hp == 0:
                        nc.vector.tensor_copy(out=o_part[:, 512 * n: 512 * (n + 1)],
                                              in_=op)
                    else:
                        nc.vector.tensor_add(out=o_all[:, 512 * n: 512 * (n + 1)],
                                             in0=op,
                                             in1=o_part[:, 512 * n: 512 * (n + 1)])
                        nc.sync.dma_start(out=out[:, 512 * n: 512 * (n + 1)],
                                          in_=o_all[:, 512 * n: 512 * (n + 1)])


# ---------------------------------------------------------------------------
# host side
# ---------------------------------------------------------------------------

def build_core_inputs(hidden_states, Wq, Wk, Wv, Wo, key_cache, value_cache):
    """Shard + lay out the full inputs into the 8 per-core DRAM images."""
    tokens = np.ascontiguousarray(hidden_states.reshape(NTOK, H))
    xT = tokens.T.astype(np.float16)                       # [4096, 32]
    xT_sb = np.ascontiguousarray(
        xT.reshape(NT, 128, NTOK).transpose(1, 0, 2)).reshape(128, NT * NTOK)

    WqT = Wq.T.astype(np.float16)                          # [in=4096, out=4096]
    WkT = Wk.T.astype(np.float16)
    WvT = (Wv.T * np.float32(ALPHA)).astype(np.float16)    # ALPHA folded into v_new
    WoT = Wo.T.astype(np.float16)                          # [in, out]
    Kf = key_cache[:, :, :POS, :].astype(np.float32)       # [B, NH, POS, HD]
    K8p = Kf[:, :, :P8, :]                                 # int8 part
    csc = np.abs(K8p).max(axis=-1, keepdims=True) * np.float32(1.0 / 127.0)
    K8 = np.round(K8p / csc).astype(np.int8)
    KF8 = (Kf[:, :, P8:, :] * np.float32(ALPHA)).astype(E3M4)  # e3m4 part
    V8 = (value_cache[:, :, :POS, :] * np.float32(ALPHA)).astype(E3M4)

    mask = np.where(np.arange(S)[:, None] > np.arange(S)[None, :],
                    np.float32(NEG_INF), np.float32(0.0))
    ident = np.eye(32, dtype=np.float16)

    in_maps = []
    for c in range(N_CORES):
        cs = slice(KPC * c, KPC * (c + 1))
        hs = slice(HPC * c, HPC * (c + 1))

        def wlayout(WT):
            a = np.ascontiguousarray(WT[:, cs])            # [4096, 512]
            return np.ascontiguousarray(
                a.reshape(NT, 128, KPC).transpose(1, 0, 2)).reshape(128, NT * KPC)

        wo_c = np.ascontiguousarray(WoT[cs, :])            # [512, 4096]
        wo_c = np.ascontiguousarray(
            wo_c.reshape(HPC, 128, H).transpose(1, 0, 2)).reshape(128, HPC * H)

        kt_c = np.ascontiguousarray(
            K8[:, hs].transpose(0, 3, 1, 2)).reshape(B, 128, HPC * P8)
        ktf_c = np.ascontiguousarray(
            KF8[:, hs].transpose(0, 3, 1, 2)).reshape(B, 128, HPC * PF)
        v_p = V8[:, hs].reshape(B, HPC, NT, 128, HD)       # [b, h, t, kv, d]
        v_c = np.ascontiguousarray(
            v_p.transpose(0, 3, 1, 2, 4)).reshape(B, 128, HPC * NT * HD)

        # cmap[p, ((b*HPC+h)*NT + t)*S + s]: int8 tiles get SCALE * c[...],
        # e3m4 tiles get the constant SCALE / ALPHA
        c_r = (csc[:, hs, :, 0] * np.float32(SCALE)).reshape(B, HPC, NI8, 128)
        c_full = np.full((B, HPC, NT, 128), np.float32(SCALE / ALPHA))
        c_full[:, :, :NI8, :] = c_r
        c_t = np.ascontiguousarray(c_full.transpose(3, 0, 1, 2))
        cmap_c = np.broadcast_to(
            c_t[..., None], (128, B, HPC, NT, S))
        cmap_c = np.ascontiguousarray(cmap_c).reshape(
            128, B * HPC * NT * S).astype(np.float16)

        in_maps.append({
            "xT": xT_sb, "wq": wlayout(WqT), "wk": wlayout(WkT),
            "wv": wlayout(WvT), "wo": wo_c, "kt": kt_c, "ktf": ktf_c,
            "v": v_c, "cmap": cmap_c, "mask": mask, "ident": ident,
        })
    return in_maps


def numpy_core_kernel(m):
    """Numpy mirror of the device dataflow for one core (layout validation)."""
    f = np.float32
    f16 = np.float16
    xT_sb = m["xT"].astype(f)
    xT = xT_sb.reshape(128, NT, NTOK).transpose(1, 0, 2).reshape(H, NTOK)

    def unw(w):
        return w.astype(f).reshape(128, NT, KPC).transpose(1, 0, 2).reshape(H, KPC)

    qT = (unw(m["wq"]).T @ xT).astype(f16).astype(f)      # [512 feat, 32 tok]
    kT = (unw(m["wk"]).T @ xT).astype(f16).astype(f)
    vnew = (unw(m["wv"]).T @ xT).T.astype(f16).astype(f)  # [32 tok, 512 feat], x ALPHA

    attnT = np.zeros((KPC, NTOK), f)
    for b in range(B):
        for h in range(HPC):
            colsl = slice(S * b, S * b + S)
            K8bh = m["kt"][b].astype(f)[:, P8 * h: P8 * (h + 1)]     # [hd, p8] int8
            KFbh = m["ktf"][b].astype(f)[:, PF * h: PF * (h + 1)]    # [hd, pf] e3m4
            KTbh = np.concatenate([K8bh, KFbh], axis=1)              # [hd, kv]
            scoresT = KTbh.T @ qT[HD * h: HD * (h + 1), colsl]       # [kv, 4]
            coff = (b * HPC + h) * NT * S
            cm = m["cmap"][:, coff: coff + NT * S].astype(f)         # [128, NT*S]
            cm_kv = cm.reshape(128, NT, S).transpose(1, 0, 2).reshape(POS, S)
            scoresT = scoresT * cm_kv
            snew = kT[HD * h: HD * (h + 1), colsl].T @ qT[HD * h: HD * (h + 1), colsl]
            snew = snew + m["mask"]                                  # [j, s]
            pr = np.exp(scoresT).astype(f16).astype(f)
            prnew = np.exp(SCALE * snew).astype(f16).astype(f)
            den = (pr.sum(axis=0) + prnew.sum(axis=0)).astype(f16).astype(f)
            vb = m["v"][b].astype(f)[:, NT * HD * h: NT * HD * (h + 1)]  # x ALPHA
            V_bh = vb.reshape(128, NT, HD).transpose(1, 0, 2).reshape(POS, HD)
            ou = V_bh.T @ pr + vnew[S * b: S * b + S, HD * h: HD * (h + 1)].T @ prnew
            attnT[HD * h: HD * (h + 1), colsl] = (ou / (ALPHA * den)).astype(f16)
    woc = m["wo"].astype(f).reshape(128, HPC, H).transpose(1, 0, 2).reshape(KPC, H)
    return (attnT.astype(f16).astype(f).T @ woc).astype(np.float16).astype(np.float32)


_NC_CACHE = None


def get_nc():
    global _NC_CACHE
    if _NC_CACHE is None:
        _NC_CACHE = build_nc()
    return _NC_CACHE


def run_on_hw(inputs, trace=False, trace_cores=None):
    position = int(inputs["position"])
    assert position == POS, position
    in_maps = build_core_inputs(
        np.asarray(inputs["hidden_states"]), np.asarray(inputs["Wq"]),
        np.asarray(inputs["Wk"]), np.asarray(inputs["Wv"]), np.asarray(inputs["Wo"]),
        np.asarray(inputs["key_cache"]), np.asarray(inputs["value_cache"]))
    nc = get_nc()
    res = run_bass_kernel_spmd(nc, in_maps, core_ids=list(range(N_CORES)),
                               trace=trace, trace_cores=trace_cores)
    partial = np.zeros((NTOK, H), np.float64)
    for c in range(N_CORES):
        partial += res.results[c]["out"].astype(np.float64)
    out = partial.astype(np.float32).reshape(B, S, H)
    return out, res


def kernel(**inputs) -> np.ndarray:
    out, _ = run_on_hw(inputs, trace=False)
    return out
